# revision 16
# baseline (speedup 1.0000x reference)
"""Trainium2 Bass kernel for a second-order-CRF (triple-tag) forward loss.

Math (matches the reference):
    flat    = scores.reshape(S, B, T^3)
    tg      = sum_{s,b} flat[s, b, target[s,b]]                    (mask all ones)
    part_2[b,u,v]   = scores[0,b,ST,ST,u] + scores[1,b,ST,u,v]     (u=tag1, v=tag2)
    part_{t+1}[b,v,w] = logsumexp_u(part_t[b,u,v] + scores[t,b,u,v,w])   t=2..S-1
    loss    = (sum_b part_S[b,EN,EN] - tg) / B

Device formulation: exp-space recurrence with a constant per-step log-offset C
(no per-step log/exp of the state):
    D_2 = exp(part_2 - C);   D_{t+1}[b,v,w] = sum_u D_t[b,u,v] * exp(s_t[b,u,v,w] - C)
so z_b = log D_S[b,EN,EN] + (S-1)*C.  With C=4.17, log D stays in [-35, +5].

v6: the fused multiply+reduce runs as ONE custom DVE op.  MUL_SCAN_SUBDIM_ANT
is registered at import time (runtime extension of dve_ops.OPS; rows 17..31 of
the custom-DVE opcode table are free): body = inclusive prefix scan of
Src0*Src1 along the free stream, with a SUB_DIM_DONE trigger that reseeds the
accumulator at each 32-element row.  Every element is written (no write
gating), so out[:, w, 31] is sum_u E[.,w,u]*D[.,u] with full f32 internal
accumulation.  Layout: partition=(b, v), free=(w, u), u innermost (scores
host-cast to bf16, pre-transposed to [s, b, v, w, u]).  Per step:
    ACT : E = exp(raw - C)                        bf16, off the serial path
    DVE : scn = mul_scan_subdim(E, D_bcast)       1x, one pass
          D' = 32x32 block transpose(scn[:, :, 31])   (b,v)->(b,w) relabel
The gold-path energy is computed on host in float64 (0.003% of the work).
"""

import copy
import sys

import numpy as np

for _p in ("/opt/trn_rl_repo",):
    if _p not in sys.path:
        sys.path.insert(0, _p)

import ml_dtypes

import concourse.bass as bass
import concourse.bacc as bacc
import concourse.tile as tile
from concourse import mybir
from concourse import bass_utils

S = 128          # sequence length
B = 32           # full batch
NCORES = 8
BL = B // NCORES  # batch per core = 4
T = 32           # tag count
START, END = 30, 31
C_OFF = 4.17     # per-step log-space renormalization constant
# recurrence steps per DMA chunk; small first chunks so the vector engine
# starts as early as possible (DMA + exp of chunk 0 gate the first mul)
CHUNKS = [2, 4] + [6] * 20
assert sum(CHUNKS) == S - 2
F32 = mybir.dt.float32
BF16 = mybir.dt.bfloat16

_cache = {}
LAST_RESULT = None  # BassKernelResults of the most recent run (for profiling)

_SCAN_OP_NAME = "MUL_SCAN_SUBDIM_ANT"
_SCAN_OP_ROW = 17  # rows [1, 0x20) are free; 1..16 used by the stock OPS


def _register_mul_scan_op():
    """Register the custom DVE op: per-subdim-reseeded prefix scan of
    Src0*Src1.  Base lowering of scan(add, Src0*Src1) gives a 2-uop FSM
    (seed: acc <- product for 1 element; steady: acc <- acc + product until
    SRC_TENSOR_DONE).  We add a SUB_DIM_DONE trigger on the steady state that
    jumps to a clone of the seed, restarting the accumulation at each
    32-element input row.  All elements are written, so completion semantics
    are identical to the stock scan ops."""
    from concourse import dve_ops
    from concourse.dve_spec import AluOp, Spec, Src0, Src1, scan, lower
    from concourse.dve_uop import DveOpSpec, Trigger

    for op in dve_ops.OPS:
        if op.name == _SCAN_OP_NAME:
            return op

    def _ref(in0, in1, c0, c1, c2):
        x = np.asarray(in0, np.float32) * np.asarray(in1, np.float32)
        flat = x.reshape(x.shape[0], -1, T)
        return np.cumsum(flat, axis=-1).reshape(x.shape)

    spec = Spec(body=scan(AluOp.ADD, Src0 * Src1), reference=_ref)
    u_seed, u_steady = lower(spec, ver="v3")
    u_seed2 = copy.deepcopy(u_seed)          # mid-stream reseed; next=(1,0,0)
    u_steady.trigger = (Trigger.SRC_TENSOR_DONE, Trigger.SUB_DIM_DONE, Trigger.NONE)
    u_steady.next_uop = (0, 2, 0)            # done -> idle; row end -> reseed

    # 2x_1p variant: one issue slot = one packed bf16 pair.  b0 multiplies the
    # lo elements, b1 the hi elements, b2 sums the pair, b3 accumulates (seed:
    # loads ZERO, mirroring the stock 1x seed at the acc block).  Both write
    # halves carry the running sum; the extracted element (index 31, a hi
    # position) is the exact full-row accumulation.  If the engine declines
    # the mode it falls back to the 1x program at slot +0.
    from concourse.dve_uop import (
        AluInp, AluOp as UAluOp, DelayInp, InpSel, OutPath, OutSel, UopConfig,
    )

    def _mk2x(seed_blk):
        u = UopConfig()
        for lane, src in ((1, InpSel.SRC_0), (2, InpSel.SRC_1),
                          (3, InpSel.SRC_0_HI), (4, InpSel.SRC_1_HI),
                          (5, InpSel.ZERO)):
            u.enable_input(src, lane)
        dp = u.datapath_config
        dp[0].enable_alu(UAluOp.MULTIPLY, AluInp.PREV_DELAY_0, AluInp.PREV_DELAY_1)
        dp[0].pass_through_delay(2, 3, 4)
        dp[1].enable_alu(UAluOp.MULTIPLY, AluInp.PREV_DELAY_2, AluInp.PREV_DELAY_3)
        dp[1].enable_delay_from_src(DelayInp.PREV_ALU_OUT, 0)
        dp[1].pass_through_delay(4)
        dp[2].enable_alu(UAluOp.ADD, AluInp.PREV_ALU_OUT, AluInp.PREV_DELAY_0)
        dp[2].pass_through_delay(4)
        if seed_blk:
            dp[3].enable_alu(UAluOp.BYPASS, AluInp.PREV_DELAY_4)
        else:
            dp[3].enable_alu(UAluOp.ADD, AluInp.CURR_ALU_OUT, AluInp.PREV_ALU_OUT)
        for k in range(4, 8):
            dp[k].pass_through_alu()
        if not seed_blk:
            u.enable_output(OutSel.ALU_OUT, OutPath.WR0_LO)
            u.enable_output(OutSel.ALU_OUT, OutPath.WR0_HI)
        return u

    x_seed = _mk2x(True)
    x_seed.repeat_count = 1
    x_seed.trigger = (Trigger.COUNT, Trigger.NONE, Trigger.NONE)
    x_seed.next_uop = (1, 0, 0)
    x_steady = _mk2x(False)
    x_steady.trigger = (Trigger.SRC_TENSOR_DONE, Trigger.SUB_DIM_DONE, Trigger.NONE)
    x_steady.next_uop = (0, 2, 0)
    x_seed2 = copy.deepcopy(x_seed)

    opspec = DveOpSpec(
        name=_SCAN_OP_NAME, opcode=_SCAN_OP_ROW,
        uops=[u_seed, u_steady, u_seed2],
        uops_2x=[x_seed, x_steady, x_seed2], perf_max=1,
        rd1_en=True,
    )
    op = dve_ops.DveOp(name=_SCAN_OP_NAME, spec=spec, subdim=True, uops_sha={})
    dve_ops._SUB_OPCODE_FOR_NAME[_SCAN_OP_NAME] = _SCAN_OP_ROW
    dve_ops._COMPILE_CACHE[(_SCAN_OP_NAME, "v3")] = opspec
    dve_ops.OPS.append(op)
    dve_ops.CUSTOM_DVE_SPECS[_SCAN_OP_NAME] = spec
    return op


def _build_program() -> bass.Bass:
    from contextlib import ExitStack

    scan_op = _register_mul_scan_op()

    nc = bacc.Bacc("TRN2", target_bir_lowering=False)
    # scores_t: host-pretransposed bf16 shard, axes [s, b, v, w, u]
    sc = nc.dram_tensor("scores_t", [S, BL, T, T, T], BF16, kind="ExternalInput")
    # D_2 = exp(part_2 - C) precomputed on host, [(b, v), u] tile layout
    d0in = nc.dram_tensor("init_d2", [BL * T, T], BF16, kind="ExternalInput")
    dout = nc.dram_tensor("dout", [BL * T, T], F32, kind="ExternalOutput")

    SB = BL * T * T * T      # element stride between steps (131072)

    with tile.TileContext(nc) as tc, ExitStack() as ctx:
        raw = ctx.enter_context(tc.tile_pool(name="raw", bufs=2))
        epool = ctx.enter_context(tc.tile_pool(name="epool", bufs=3))
        ppool = ctx.enter_context(tc.tile_pool(name="ppool", bufs=2))
        rpool = ctx.enter_context(tc.tile_pool(name="rpool", bufs=2))
        small = ctx.enter_context(tc.tile_pool(name="small", bufs=1))

        cbias = small.tile([BL * T, 1], F32)
        nc.vector.memset(cbias[...], -C_OFF)

        d_cur = rpool.tile([BL * T, T], BF16)
        nc.sync.dma_start(out=d_cur[...], in_=d0in[...])

        s0 = 2
        for ch in CHUNKS:
            rawt = raw.tile([BL * T, ch, T, T], BF16)
            nc.sync.dma_start(
                out=rawt[...],
                in_=bass.AP(
                    tensor=sc[...].tensor,
                    offset=s0 * SB,
                    ap=[[T * T, BL * T], [SB, ch], [T, T], [1, T]],
                ),
            )
            et = epool.tile([BL * T, ch, T, T], BF16)
            nc.scalar.activation(
                out=et[...], in_=rawt[...],
                func=mybir.ActivationFunctionType.Exp, bias=cbias[...],
            )
            for j in range(ch):
                t_idx = s0 + j
                # scn[(b,v), w, u] = running sum_u' <= u of E[.,w,u']*D[.,u'],
                # reseeded at each w row: scn[:, w, T-1] = sum_u E*D
                scn = ppool.tile([BL * T, T, T], BF16)
                inst = nc.vector._custom_dve(
                    scan_op,
                    out=scn[...],
                    in0=et[:, j],
                    in1=d_cur[...].unsqueeze(1).broadcast_to([BL * T, T, T]),
                )
                # byte-36[7:6]: allow the engine to reach the 2x_1p slot
                (inst.inst if hasattr(inst, "inst") else inst).perf_max = 1
                if t_idx < S - 1:
                    d_nxt = rpool.tile([BL * T, T], BF16)
                    nc.vector.transpose(out=d_nxt[...], in_=scn[:, :, T - 1])
                    d_cur = d_nxt
                else:
                    redf = rpool.tile([BL * T, T], F32)
                    nc.vector.tensor_copy(out=redf[...], in_=scn[:, :, T - 1])
                    nc.sync.dma_start(out=dout[...], in_=redf[...])
            s0 += ch
    nc.compile()
    return nc


def _get_program() -> bass.Bass:
    if "nc" not in _cache:
        _cache["nc"] = _build_program()
    return _cache["nc"]


def kernel(scores, target, mask=None, **_unused):
    scores = np.asarray(scores, dtype=np.float32)
    target = np.asarray(target)

    # gold-path energy on host (f64): 0.003% of the work, better precision
    flat = scores.reshape(S, B, T * T * T)
    tgt = target.reshape(S, B).astype(np.int64)
    tg_energy = np.take_along_axis(flat, tgt[:, :, None], axis=2)[..., 0]
    if mask is not None:
        tg_energy = np.where(np.asarray(mask, dtype=bool), tg_energy, 0.0)
    total_tg = tg_energy.astype(np.float64).sum()

    # [s, b, u, v, w] -> [s, b, v, w, u] so each step tile is one contiguous
    # 256 KiB bf16 DMA with partition=(b,v), free=(w,u), u innermost.
    sct = np.ascontiguousarray(
        scores.transpose(0, 1, 3, 4, 2).astype(ml_dtypes.bfloat16)
    )

    # D_2[(b,v), u] = exp(part_2[b,u,v] - C)
    p1 = scores[0, :, START, START, :]               # (B, tag1=u)
    s1 = scores[1, :, START, :, :]                   # (B, u, v)
    part2 = p1[:, :, None] + s1                      # (B, u, v)
    d2 = np.exp(part2 - C_OFF).transpose(0, 2, 1)    # (B, v, u)

    nc = _get_program()
    in_maps = []
    for core in range(NCORES):
        bs = slice(core * BL, (core + 1) * BL)
        in_maps.append({
            "scores_t": np.ascontiguousarray(sct[:, bs]),
            "init_d2": np.ascontiguousarray(
                d2[bs].reshape(BL * T, T).astype(ml_dtypes.bfloat16)
            ),
        })

    res = bass_utils.run_bass_kernel_spmd(nc, in_maps, core_ids=list(range(NCORES)))
    global LAST_RESULT
    LAST_RESULT = res

    total_z = 0.0
    for core in range(NCORES):
        out = res.results[core]
        d_end = out["dout"][T - 1 :: T, END].astype(np.float64)  # D_S[b, END, END]
        total_z += (np.log(d_end) + (S - 1) * C_OFF).sum()
    return np.asarray((total_z - total_tg) / B, dtype=np.float32)
